# revision 6
# baseline (speedup 1.0000x reference)
"""Full (non-causal) multi-head attention for Trainium2, 8-core SPMD.

Problem: B=4, L=2048, H=16, E=64 fp32.
  scores = einsum('blhe,bshe->bhls', Q, K) * 1/sqrt(E)
  attn   = softmax(scores, axis=-1)
  out    = einsum('bhls,bshd->blhd', attn, V)

Sharding: the 64 (b,h) pairs are split over 8 NeuronCores, 8 pairs per
core; attention is fully independent per (b,h), so no cross-core
communication.  The host hands each core Q^T/K^T/V already transposed /
cast to bf16 so DMA lands them ready for the PE, and takes back an
unnormalized O'[e+1, l] per pair — the softmax denominator ride-along
row — dividing + final transpose on the host (0.1% of the FLOPs).

Per-core algorithm (per (b,h) pair):
  - Scores are computed transposed, S^T[s, l], in [128, 2*LQ] PSUM tiles
    (chunk c, l-halves), QK matmuls 2-way row-tiled (contraction E=64)
    so even/odd chunks run concurrently on the two PE row halves.
  - l loops in halves of 2*LQ=1024 so K^T and V stationary tiles are
    loaded once per half instead of once per l-quarter.
  - exp() is split across two engines, working different PSUM banks
    concurrently: ScalarE runs exact exp (activation, bf16 out) on 11
    of 16 chunks; the DVE computes the other 5 with a mean-centered
    Schraudolph bit-trick exp (i16 = round(score*A + B), bitcast bf16;
    sigma ~1.8% per element, mean error ~0, common mode cancels in the
    softmax divide).
  - AV accumulates O'[e+1, l] over s-chunks in PSUM, all-bf16; V
    carries a ones column so row 64 of O' is the softmax denominator.
  - Dummy warmup matmuls at kernel start keep the PE HAM activity
    window busy during the first DMA wait so real matmuls start at
    2.4 GHz instead of 1.2 GHz.
"""

import math

import numpy as np
import ml_dtypes
from contextlib import ExitStack

import concourse.bass as bass
import concourse.mybir as mybir
import concourse.tile as tile
from concourse import bacc
from concourse.bass_utils import run_bass_kernel_spmd

N_CORES = 8
B, L, H, E = 4, 2048, 16, 64
PAIRS = (B * H) // N_CORES    # 8 (b,h) pairs per core
P = 128                       # s-chunk size / partition count
NCHUNK = L // P               # 16 s-chunks
LQ = 512                      # max fp32 matmul free dim (one PSUM bank)
LH = 2 * LQ                   # l-half processed per pass
NPASS = L // LH               # 2 passes over l per pair
SCALE = 1.0 / 8.0             # 1/sqrt(E)

F32 = mybir.dt.float32
BF16 = mybir.dt.bfloat16
I16 = mybir.dt.int16

# Schraudolph constants: i16 = round(score*SA + SB) bitcast bf16
# approximates exp(score/8) with zero-mean multiplicative error.
LOG2E = 1.4426950408889634
SA = 128.0 * LOG2E * SCALE
_fs = np.linspace(0.0, 1.0, 200001)[:-1]
_ECORR = float(np.mean((1.0 + _fs) * 2.0 ** (-_fs)))
SB = 127.0 * 128.0 - 128.0 * math.log2(_ECORR)

# Per chunk the score tile spans 2 PSUM banks (2 l-quarters): ScalarE
# always takes bank 0 (exact exp); bank 1 goes to the DVE (Schraudolph)
# except for SCALAR_FULL chunks where ScalarE does both (load balance).
SCALAR_FULL = (0, 6, 12)
LAG = 4                       # AV trails QK by this many chunks
N_WARMUP = 40


def _attention(tc: tile.TileContext, o, qt, kt_d, v):
    nc = tc.nc
    EXPF = mybir.ActivationFunctionType.Exp

    with ExitStack() as ctx:
        # PE warmup: dummy matmuls on a zero tile keep the HAM activity
        # window busy while the first pair's DMAs land.
        with tc.tile_pool(name="wsb", bufs=1) as wsb, \
             tc.tile_pool(name="wps", bufs=1, space="PSUM") as wps:
            wz = wsb.tile([64, 64], BF16, tag="wz")
            nc.vector.memset(wz[:], 0.0)
            wp = wps.tile([64, 64], F32, tag="wp")
            for _ in range(N_WARMUP):
                nc.tensor.matmul(wp[:], wz[:], wz[:], start=True, stop=True)

        io = ctx.enter_context(tc.tile_pool(name="io", bufs=2))
        etp = ctx.enter_context(tc.tile_pool(name="etp", bufs=LAG + 2))
        osb = ctx.enter_context(tc.tile_pool(name="osb", bufs=2))

        # PSUM: score 3x2 banks + oacc 1x2 = 8
        pscore = ctx.enter_context(tc.tile_pool(name="pscore", bufs=3, space="PSUM"))
        pacc = ctx.enter_context(tc.tile_pool(name="pacc", bufs=1, space="PSUM"))

        for p in range(PAIRS):
            # ---- load Q^T (duplicated to both halves), K^T (paired), V ----
            qtd = io.tile([P, L], BF16, tag="qtd")
            nc.sync.dma_start(out=qtd[0:E, :], in_=qt[p])
            nc.sync.dma_start(out=qtd[E:P, :], in_=qt[p])

            # kt_d[p] is [2, 8, 64, 128]: half h holds chunks 2c+h.
            kt = io.tile([P, NCHUNK // 2, P], BF16, tag="kt")
            nc.sync.dma_start(
                out=kt[0:E, :, :], in_=kt_d[p, 0].rearrange("c e l -> e c l")
            )
            nc.sync.dma_start(
                out=kt[E:P, :, :], in_=kt_d[p, 1].rearrange("c e l -> e c l")
            )

            vp = io.tile([P, NCHUNK, E + 1], BF16, tag="vp")
            nc.sync.dma_start(
                out=vp[:, :, 0:E], in_=v[p].rearrange("(c p) e -> p c e", p=P)
            )
            nc.vector.memset(vp[:, :, E : E + 1], 1.0)

            # ---- main loop: scores^T -> exp -> AV, l in halves of 1024 ----
            # Software-pipelined: AV trails QK by LAG chunks so a stalled
            # AV matmul (waiting on exp) never starves the in-order PE
            # queue of QK work; et ring buffers carry exp output forward.
            osum = osb.tile([E + 1, L], F32, tag="osum")
            for h in range(NPASS):
                oacc = pacc.tile([E + 1, LH], F32, tag="oacc")
                hsl = slice(h * LH, (h + 1) * LH)
                ets = [None] * NCHUNK
                for c in range(NCHUNK + LAG):
                    if c < NCHUNK:
                        half = c % 2
                        lo, hi = (0, E) if half == 0 else (E, P)
                        score = pscore.tile([P, LH], F32, tag="score")
                        for j in range(2):
                            qsl = slice((h * 2 + j) * LQ, (h * 2 + j + 1) * LQ)
                            nc.tensor.matmul(
                                score[:, j * LQ : (j + 1) * LQ],
                                kt[lo:hi, c // 2, :],
                                qtd[lo:hi, qsl],
                                start=True, stop=True,
                                tile_position=(lo, 0),
                            )
                        et = etp.tile([P, LH], BF16, tag="et")
                        ets[c] = et
                        # exact exp on ScalarE: bank 0 (+ bank 1 on
                        # SCALAR_FULL chunks); Schraudolph on DVE: bank 1
                        if c in SCALAR_FULL:
                            nc.scalar.activation(et[:], score[:], EXPF,
                                                 scale=SCALE)
                        else:
                            nc.scalar.activation(
                                et[:, 0:LQ], score[:, 0:LQ], EXPF, scale=SCALE)
                            nc.vector.tensor_scalar(
                                et[:, LQ:LH].bitcast(I16), score[:, LQ:LH],
                                float(SA), float(SB),
                                mybir.AluOpType.mult, mybir.AluOpType.add,
                            )
                    if c >= LAG:
                        cc = c - LAG
                        # AV accumulate: O'[e+1, l] += V'^T_chunk @ E_chunk
                        for j in range(2):
                            nc.tensor.matmul(
                                oacc[:, j * LQ : (j + 1) * LQ],
                                vp[:, cc, :],
                                ets[cc][:, j * LQ : (j + 1) * LQ],
                                start=(cc == 0), stop=(cc == NCHUNK - 1),
                            )
                nc.vector.tensor_copy(osum[:, hsl], oacc[:])

            nc.sync.dma_start(out=o[p], in_=osum[:])


_CACHE = {}


def _build():
    if "nc" in _CACHE:
        return _CACHE["nc"]
    nc = bacc.Bacc("TRN2", target_bir_lowering=False, debug=False,
                   num_devices=N_CORES)
    qt = nc.dram_tensor("qt", [PAIRS, E, L], BF16, kind="ExternalInput").ap()
    kt = nc.dram_tensor("kt", [PAIRS, 2, NCHUNK // 2, E, P], BF16,
                        kind="ExternalInput").ap()
    v = nc.dram_tensor("v", [PAIRS, L, E], BF16, kind="ExternalInput").ap()
    o = nc.dram_tensor("o", [PAIRS, E + 1, L], F32, kind="ExternalOutput").ap()
    with tile.TileContext(nc) as tc:
        _attention(tc, o, qt, kt, v)
    nc.compile()
    _CACHE["nc"] = nc
    return nc


def run(queries, keys, values, trace=False, **kw):
    """Run the SPMD kernel; returns (out_full, BassKernelResults)."""
    nc = _build()
    # [B, L, H, E] -> heads-major layouts the device DMAs straight in.
    qh = np.transpose(np.asarray(queries), (0, 2, 3, 1)).reshape(B * H, E, L)
    qh = np.ascontiguousarray(qh).astype(ml_dtypes.bfloat16)   # [64, E, L]
    kh = np.transpose(np.asarray(keys), (0, 2, 3, 1)).reshape(B * H, E, L)
    # [64, E, L] -> [64, 2, 8, E, 128]: half h gets s-chunks 2c+h
    kh = kh.reshape(B * H, E, NCHUNK // 2, 2, P)
    kh = np.ascontiguousarray(np.transpose(kh, (0, 3, 2, 1, 4))).astype(
        ml_dtypes.bfloat16)
    vh = np.transpose(np.asarray(values), (0, 2, 1, 3)).reshape(B * H, L, E)
    vh = np.ascontiguousarray(vh).astype(ml_dtypes.bfloat16)
    in_maps = [
        {"qt": qh[c * PAIRS : (c + 1) * PAIRS],
         "kt": kh[c * PAIRS : (c + 1) * PAIRS],
         "v": vh[c * PAIRS : (c + 1) * PAIRS]}
        for c in range(N_CORES)
    ]
    res = run_bass_kernel_spmd(nc, in_maps, list(range(N_CORES)),
                               trace=trace, **kw)
    # [64, E+1, L]: rows 0..63 unnormalized O^T, row 64 the softmax sums
    oh = np.concatenate([res.results[c]["o"] for c in range(N_CORES)], axis=0)
    onorm = oh[:, 0:E, :] / oh[:, E : E + 1, :]          # softmax divide
    out = np.transpose(onorm.reshape(B, H, E, L), (0, 3, 1, 2))
    return np.ascontiguousarray(out), res


def kernel(queries, keys, values):
    out, _ = run(queries, keys, values)
    return out


# revision 9
# speedup vs baseline: 1.2288x; 1.2288x over previous
"""Full (non-causal) multi-head attention for Trainium2, 8-core SPMD.

Problem: B=4, L=2048, H=16, E=64 fp32.
  scores = einsum('blhe,bshe->bhls', Q, K) * 1/sqrt(E)
  attn   = softmax(scores, axis=-1)
  out    = einsum('bhls,bshd->blhd', attn, V)

Sharding: the 64 (b,h) pairs are split over 8 NeuronCores, 8 pairs per
core; attention is fully independent per (b,h), so no cross-core
communication.  The host hands each core Q^T/K^T/V already transposed /
cast to bf16 so DMA lands them ready for the PE, and takes back an
unnormalized O'[e+1, l] per pair — the softmax denominator ride-along
row — dividing + final transpose on the host (0.1% of the FLOPs).

Per-core algorithm (per (b,h) pair):
  - Scores are computed transposed, S^T[s, l], in [128, 2*LQ] PSUM tiles
    (chunk c, l-halves), QK matmuls 2-way row-tiled (contraction E=64)
    so even/odd chunks run concurrently on the two PE row halves.
  - l loops in halves of 2*LQ=1024 so K^T and V stationary tiles are
    loaded once per half instead of once per l-quarter.
  - exp() is split across two engines, working different PSUM banks
    concurrently: ScalarE runs exact exp (activation, bf16 out) on 11
    of 16 chunks; the DVE computes the other 5 with a mean-centered
    Schraudolph bit-trick exp (i16 = round(score*A + B), bitcast bf16;
    sigma ~1.8% per element, mean error ~0, common mode cancels in the
    softmax divide).
  - AV accumulates O'[e+1, l] over s-chunks in PSUM, all-bf16; V
    carries a ones column so row 64 of O' is the softmax denominator.
  - Dummy warmup matmuls at kernel start keep the PE HAM activity
    window busy during the first DMA wait so real matmuls start at
    2.4 GHz instead of 1.2 GHz.
"""

import math

import numpy as np
import ml_dtypes
from contextlib import ExitStack

import concourse.bass as bass
import concourse.mybir as mybir
import concourse.tile as tile
from concourse import bacc
from concourse.bass_utils import run_bass_kernel_spmd

N_CORES = 8
B, L, H, E = 4, 2048, 16, 64
PAIRS = (B * H) // N_CORES    # 8 (b,h) pairs per core
P = 128                       # s-chunk size / partition count
NCHUNK = L // P               # 16 s-chunks
LQ = 512                      # max fp32 matmul free dim (one PSUM bank)
LH = 2 * LQ                   # l-half processed per pass
NPASS = L // LH               # 2 passes over l per pair
SCALE = 1.0 / 8.0             # 1/sqrt(E)

F32 = mybir.dt.float32
BF16 = mybir.dt.bfloat16
I16 = mybir.dt.int16

# Schraudolph constants: i16 = round(score*SA + SB) bitcast bf16
# approximates exp(score/8) with zero-mean multiplicative error.
LOG2E = 1.4426950408889634
SA = 128.0 * LOG2E * SCALE
_fs = np.linspace(0.0, 1.0, 200001)[:-1]
_ECORR = float(np.mean((1.0 + _fs) * 2.0 ** (-_fs)))
SB = 127.0 * 128.0 - 128.0 * math.log2(_ECORR)

# Per chunk there are two single-bank score tiles (l-quarters j=0,1):
# ScalarE always takes j=0 (exact exp); j=1 goes to the DVE (Schraudolph)
# except for SCALAR_FULL chunks where ScalarE does both (load balance).
SCALAR_FULL = (0, 6, 12)
LAG = 4                       # AV trails QK by this many chunks (even)
N_WARMUP = 40


def _attention(tc: tile.TileContext, o, qt, kt_d, v):
    nc = tc.nc
    EXPF = mybir.ActivationFunctionType.Exp

    with ExitStack() as ctx:
        # PE warmup: dummy matmuls on a zero tile keep the HAM activity
        # window busy while the first pair's DMAs land.
        with tc.tile_pool(name="wsb", bufs=1) as wsb, \
             tc.tile_pool(name="wps", bufs=1, space="PSUM") as wps:
            wz = wsb.tile([64, 64], BF16, tag="wz")
            nc.vector.memset(wz[:], 0.0)
            wp = wps.tile([64, 64], F32, tag="wp")
            for _ in range(N_WARMUP):
                nc.tensor.matmul(wp[:], wz[:], wz[:], start=True, stop=True)

        io = ctx.enter_context(tc.tile_pool(name="io", bufs=2))
        etp = ctx.enter_context(tc.tile_pool(name="etp", bufs=LAG + 2))
        osb = ctx.enter_context(tc.tile_pool(name="osb", bufs=2))

        # PSUM: score 6x1 banks + oacc 1x2 = 8
        pscore = ctx.enter_context(tc.tile_pool(name="pscore", bufs=6, space="PSUM"))
        pacc = ctx.enter_context(tc.tile_pool(name="pacc", bufs=1, space="PSUM"))

        for p in range(PAIRS):
            # ---- load Q^T (duplicated to both halves), K^T (paired), V ----
            qtd = io.tile([P, L], BF16, tag="qtd")
            nc.sync.dma_start(out=qtd[0:E, :], in_=qt[p])
            nc.sync.dma_start(out=qtd[E:P, :], in_=qt[p])

            # kt_d[p] is [2, 8, 64, 128]: half h holds chunks 2c+h.
            kt = io.tile([P, NCHUNK // 2, P], BF16, tag="kt")
            nc.sync.dma_start(
                out=kt[0:E, :, :], in_=kt_d[p, 0].rearrange("c e l -> e c l")
            )
            nc.sync.dma_start(
                out=kt[E:P, :, :], in_=kt_d[p, 1].rearrange("c e l -> e c l")
            )

            vp = io.tile([P, NCHUNK, E + 1], BF16, tag="vp")
            nc.sync.dma_start(
                out=vp[:, :, 0:E], in_=v[p].rearrange("(c p) e -> p c e", p=P)
            )
            nc.vector.memset(vp[:, :, E : E + 1], 1.0)

            # ---- main loop: scores^T -> exp -> AV, l in halves of 1024 ----
            # Software-pipelined in 2-chunk bursts: AV trails QK by LAG
            # chunks so a stalled AV matmul (waiting on exp) never starves
            # the in-order PE queue; et ring buffers carry exp forward.
            # Within a burst the even/odd chunks run on the two PE row
            # halves concurrently, and the second l-quarter matmul of each
            # stationary sets ldweights=False to skip the redundant reload.
            osum = osb.tile([E + 1, L], F32, tag="osum")
            for h in range(NPASS):
                oacc = pacc.tile([E + 1, LH], F32, tag="oacc")
                hsl = slice(h * LH, (h + 1) * LH)
                ets = [None] * NCHUNK
                for m in range(0, NCHUNK + LAG, 2):
                    burst = [c for c in (m, m + 1) if c < NCHUNK]
                    for c in burst:
                        lo, hi = (0, E) if c % 2 == 0 else (E, P)
                        scores = []
                        for j in range(2):
                            qsl = slice((h * 2 + j) * LQ, (h * 2 + j + 1) * LQ)
                            sc = pscore.tile([P, LQ], F32, tag="score")
                            scores.append(sc)
                            mm = nc.tensor.matmul(
                                sc[:], kt[lo:hi, c // 2, :], qtd[lo:hi, qsl],
                                start=True, stop=True,
                                tile_position=(lo, 0),
                            )
                            if j == 1:
                                mm.ins.ldweights = False
                        et = etp.tile([P, LH], BF16, tag="et")
                        ets[c] = et
                        # exact exp on ScalarE: j=0 (+ j=1 on SCALAR_FULL
                        # chunks); Schraudolph on the DVE: j=1
                        nc.scalar.activation(
                            et[:, 0:LQ], scores[0][:], EXPF, scale=SCALE)
                        if c in SCALAR_FULL:
                            nc.scalar.activation(
                                et[:, LQ:LH], scores[1][:], EXPF, scale=SCALE)
                        else:
                            nc.vector.tensor_scalar(
                                et[:, LQ:LH].bitcast(I16), scores[1][:],
                                float(SA), float(SB),
                                mybir.AluOpType.mult, mybir.AluOpType.add,
                            )
                    for c in (m - LAG, m - LAG + 1):
                        if not (0 <= c < NCHUNK):
                            continue
                        # AV accumulate: O'[e+1, l] += V'^T_chunk @ E_chunk
                        for j in range(2):
                            mm = nc.tensor.matmul(
                                oacc[:, j * LQ : (j + 1) * LQ],
                                vp[:, c, :],
                                ets[c][:, j * LQ : (j + 1) * LQ],
                                start=(c == 0), stop=(c == NCHUNK - 1),
                            )
                            if j == 1:
                                mm.ins.ldweights = False
                nc.vector.tensor_copy(osum[:, hsl], oacc[:])

            nc.sync.dma_start(out=o[p], in_=osum[:])


_CACHE = {}


def _build():
    if "nc" in _CACHE:
        return _CACHE["nc"]
    nc = bacc.Bacc("TRN2", target_bir_lowering=False, debug=False,
                   num_devices=N_CORES)
    qt = nc.dram_tensor("qt", [PAIRS, E, L], BF16, kind="ExternalInput").ap()
    kt = nc.dram_tensor("kt", [PAIRS, 2, NCHUNK // 2, E, P], BF16,
                        kind="ExternalInput").ap()
    v = nc.dram_tensor("v", [PAIRS, L, E], BF16, kind="ExternalInput").ap()
    o = nc.dram_tensor("o", [PAIRS, E + 1, L], F32, kind="ExternalOutput").ap()
    with tile.TileContext(nc) as tc:
        _attention(tc, o, qt, kt, v)
    nc.compile()
    _CACHE["nc"] = nc
    return nc


def run(queries, keys, values, trace=False, **kw):
    """Run the SPMD kernel; returns (out_full, BassKernelResults)."""
    nc = _build()
    # [B, L, H, E] -> heads-major layouts the device DMAs straight in.
    qh = np.transpose(np.asarray(queries), (0, 2, 3, 1)).reshape(B * H, E, L)
    qh = np.ascontiguousarray(qh).astype(ml_dtypes.bfloat16)   # [64, E, L]
    kh = np.transpose(np.asarray(keys), (0, 2, 3, 1)).reshape(B * H, E, L)
    # [64, E, L] -> [64, 2, 8, E, 128]: half h gets s-chunks 2c+h
    kh = kh.reshape(B * H, E, NCHUNK // 2, 2, P)
    kh = np.ascontiguousarray(np.transpose(kh, (0, 3, 2, 1, 4))).astype(
        ml_dtypes.bfloat16)
    vh = np.transpose(np.asarray(values), (0, 2, 1, 3)).reshape(B * H, L, E)
    vh = np.ascontiguousarray(vh).astype(ml_dtypes.bfloat16)
    in_maps = [
        {"qt": qh[c * PAIRS : (c + 1) * PAIRS],
         "kt": kh[c * PAIRS : (c + 1) * PAIRS],
         "v": vh[c * PAIRS : (c + 1) * PAIRS]}
        for c in range(N_CORES)
    ]
    res = run_bass_kernel_spmd(nc, in_maps, list(range(N_CORES)),
                               trace=trace, **kw)
    # [64, E+1, L]: rows 0..63 unnormalized O^T, row 64 the softmax sums
    oh = np.concatenate([res.results[c]["o"] for c in range(N_CORES)], axis=0)
    onorm = oh[:, 0:E, :] / oh[:, E : E + 1, :]          # softmax divide
    out = np.transpose(onorm.reshape(B, H, E, L), (0, 3, 1, 2))
    return np.ascontiguousarray(out), res


def kernel(queries, keys, values):
    out, _ = run(queries, keys, values)
    return out


# revision 11
# speedup vs baseline: 1.3077x; 1.0642x over previous
"""Full (non-causal) multi-head attention for Trainium2, 8-core SPMD.

Problem: B=4, L=2048, H=16, E=64 fp32.
  scores = einsum('blhe,bshe->bhls', Q, K) * 1/sqrt(E)
  attn   = softmax(scores, axis=-1)
  out    = einsum('bhls,bshd->blhd', attn, V)

Sharding: the 64 (b,h) pairs are split over 8 NeuronCores, 8 pairs per
core; attention is fully independent per (b,h), so no cross-core
communication.  The host hands each core Q^T/K^T/V already transposed /
cast to bf16 so DMA lands them ready for the PE, and takes back an
unnormalized O'[e+1, l] per pair — the softmax denominator ride-along
row — dividing + final transpose on the host (0.1% of the FLOPs).

Per-core algorithm (per (b,h) pair):
  - Scores are computed transposed, S^T[s, l], in [128, 2*LQ] PSUM tiles
    (chunk c, l-halves), QK matmuls 2-way row-tiled (contraction E=64)
    so even/odd chunks run concurrently on the two PE row halves.
  - l loops in halves of 2*LQ=1024 so K^T and V stationary tiles are
    loaded once per half instead of once per l-quarter.
  - exp() is split across two engines, working different PSUM banks
    concurrently: ScalarE runs exact exp (activation, bf16 out) on 11
    of 16 chunks; the DVE computes the other 5 with a mean-centered
    Schraudolph bit-trick exp (i16 = round(score*A + B), bitcast bf16;
    sigma ~1.8% per element, mean error ~0, common mode cancels in the
    softmax divide).
  - AV accumulates O'[e+1, l] over s-chunks in PSUM, all-bf16; V
    carries a ones column so row 64 of O' is the softmax denominator.
  - Dummy warmup matmuls at kernel start keep the PE HAM activity
    window busy during the first DMA wait so real matmuls start at
    2.4 GHz instead of 1.2 GHz.
"""

import math

import numpy as np
import ml_dtypes
from contextlib import ExitStack

import concourse.bass as bass
import concourse.mybir as mybir
import concourse.tile as tile
from concourse import bacc
from concourse.bass_utils import run_bass_kernel_spmd

N_CORES = 8
B, L, H, E = 4, 2048, 16, 64
PAIRS = (B * H) // N_CORES    # 8 (b,h) pairs per core
P = 128                       # s-chunk size / partition count
NCHUNK = L // P               # 16 s-chunks
LQ = 512                      # max fp32 matmul free dim (one PSUM bank)
LH = 2 * LQ                   # l-half processed per pass
NPASS = L // LH               # 2 passes over l per pair
SCALE = 1.0 / 8.0             # 1/sqrt(E)

F32 = mybir.dt.float32
BF16 = mybir.dt.bfloat16
I16 = mybir.dt.int16

# Schraudolph constants: i16 = round(score*SA + SB) bitcast bf16
# approximates exp(score/8) with zero-mean multiplicative error.
LOG2E = 1.4426950408889634
SA = 128.0 * LOG2E * SCALE
_fs = np.linspace(0.0, 1.0, 200001)[:-1]
_ECORR = float(np.mean((1.0 + _fs) * 2.0 ** (-_fs)))
SB = 127.0 * 128.0 - 128.0 * math.log2(_ECORR)

# Per chunk there are two single-bank score tiles (l-quarters j=0,1):
# ScalarE always takes j=0 (exact exp); j=1 goes to the DVE (Schraudolph)
# except for SCALAR_FULL chunks where ScalarE does both (load balance).
SCALAR_FULL = (0, 6, 12)
LAG = 4                       # AV trails QK by this many chunks (even)
N_WARMUP = 40


def _attention(tc: tile.TileContext, o, qt, kt_d, v):
    nc = tc.nc
    EXPF = mybir.ActivationFunctionType.Exp

    with ExitStack() as ctx:
        # PE warmup: dummy matmuls on a zero tile keep the HAM activity
        # window busy while the first pair's DMAs land.
        with tc.tile_pool(name="wsb", bufs=1) as wsb, \
             tc.tile_pool(name="wps", bufs=1, space="PSUM") as wps:
            wz = wsb.tile([64, 64], BF16, tag="wz")
            nc.vector.memset(wz[:], 0.0)
            wp = wps.tile([64, 64], F32, tag="wp")
            for _ in range(N_WARMUP):
                nc.tensor.matmul(wp[:], wz[:], wz[:], start=True, stop=True)

        io = ctx.enter_context(tc.tile_pool(name="io", bufs=2))
        etp = ctx.enter_context(tc.tile_pool(name="etp", bufs=LAG + 4))
        osb = ctx.enter_context(tc.tile_pool(name="osb", bufs=2))

        # PSUM: score 6x1 banks + oacc 1x2 = 8
        pscore = ctx.enter_context(tc.tile_pool(name="pscore", bufs=6, space="PSUM"))
        pacc = ctx.enter_context(tc.tile_pool(name="pacc", bufs=1, space="PSUM"))

        for p in range(PAIRS):
            # ---- load Q^T (duplicated to both halves), K^T (paired), V ----
            qtd = io.tile([P, L], BF16, tag="qtd")
            nc.sync.dma_start(out=qtd[0:E, :], in_=qt[p])
            nc.sync.dma_start(out=qtd[E:P, :], in_=qt[p])

            # kt_d[p] is [2, 8, 64, 128]: half h holds chunks 2c+h.
            kt = io.tile([P, NCHUNK // 2, P], BF16, tag="kt")
            nc.sync.dma_start(
                out=kt[0:E, :, :], in_=kt_d[p, 0].rearrange("c e l -> e c l")
            )
            nc.sync.dma_start(
                out=kt[E:P, :, :], in_=kt_d[p, 1].rearrange("c e l -> e c l")
            )

            vp = io.tile([P, NCHUNK, E + 1], BF16, tag="vp")
            nc.sync.dma_start(
                out=vp[:, :, 0:E], in_=v[p].rearrange("(c p) e -> p c e", p=P)
            )
            nc.vector.memset(vp[:, :, E : E + 1], 1.0)

            # ---- main loop: scores^T -> exp -> AV, l in halves of 1024 ----
            # Software-pipelined in 2-chunk bursts: AV trails QK by LAG
            # chunks so a stalled AV matmul (waiting on exp) never starves
            # the in-order PE queue; et ring buffers carry exp forward.
            # Within a burst the even/odd chunks run on the two PE row
            # halves concurrently, and the second l-quarter matmul of each
            # stationary sets ldweights=False to skip the redundant reload.
            for h in range(NPASS):
                oacc = pacc.tile([E + 1, LH], F32, tag="oacc")
                hsl = slice(h * LH, (h + 1) * LH)
                ets = [None] * NCHUNK
                for m in range(0, NCHUNK + LAG, 2):
                    burst = [c for c in (m, m + 1) if c < NCHUNK]
                    # QK interleaved across the chunk pair (c0j0, c1j0,
                    # c0j1, c1j1) so each LDWEIGHTS loads into its row
                    # half while the other half's matmul streams.
                    scs = {}
                    for j in range(2):
                        for c in burst:
                            lo, hi = (0, E) if c % 2 == 0 else (E, P)
                            qsl = slice((h * 2 + j) * LQ, (h * 2 + j + 1) * LQ)
                            sc = pscore.tile([P, LQ], F32, tag="score")
                            scs[(c, j)] = sc
                            nc.tensor.matmul(
                                sc[:], kt[lo:hi, c // 2, :], qtd[lo:hi, qsl],
                                start=True, stop=True,
                                tile_position=(lo, 0),
                            )
                    for c in burst:
                        et = etp.tile([P, LH], BF16, tag="et")
                        ets[c] = et
                        # exact exp on ScalarE: j=0 (+ j=1 on SCALAR_FULL
                        # chunks); Schraudolph on the DVE: j=1
                        nc.scalar.activation(
                            et[:, 0:LQ], scs[(c, 0)][:], EXPF, scale=SCALE)
                        if c in SCALAR_FULL:
                            nc.scalar.activation(
                                et[:, LQ:LH], scs[(c, 1)][:], EXPF, scale=SCALE)
                        else:
                            nc.vector.tensor_scalar(
                                et[:, LQ:LH].bitcast(I16), scs[(c, 1)][:],
                                float(SA), float(SB),
                                mybir.AluOpType.mult, mybir.AluOpType.add,
                            )
                    for c in (m - LAG, m - LAG + 1):
                        if not (0 <= c < NCHUNK):
                            continue
                        # AV accumulate: O'[e+1, l] += V'^T_chunk @ E_chunk
                        for j in range(2):
                            nc.tensor.matmul(
                                oacc[:, j * LQ : (j + 1) * LQ],
                                vp[:, c, :],
                                ets[c][:, j * LQ : (j + 1) * LQ],
                                start=(c == 0), stop=(c == NCHUNK - 1),
                            )
                osum = osb.tile([E + 1, LH], F32, tag="osum")
                nc.vector.tensor_copy(osum[:], oacc[:])
                nc.sync.dma_start(out=o[p][:, hsl], in_=osum[:])


_CACHE = {}


def _build():
    if "nc" in _CACHE:
        return _CACHE["nc"]
    nc = bacc.Bacc("TRN2", target_bir_lowering=False, debug=False,
                   num_devices=N_CORES)
    qt = nc.dram_tensor("qt", [PAIRS, E, L], BF16, kind="ExternalInput").ap()
    kt = nc.dram_tensor("kt", [PAIRS, 2, NCHUNK // 2, E, P], BF16,
                        kind="ExternalInput").ap()
    v = nc.dram_tensor("v", [PAIRS, L, E], BF16, kind="ExternalInput").ap()
    o = nc.dram_tensor("o", [PAIRS, E + 1, L], F32, kind="ExternalOutput").ap()
    with tile.TileContext(nc) as tc:
        _attention(tc, o, qt, kt, v)
    nc.compile()
    _CACHE["nc"] = nc
    return nc


def run(queries, keys, values, trace=False, **kw):
    """Run the SPMD kernel; returns (out_full, BassKernelResults)."""
    nc = _build()
    # [B, L, H, E] -> heads-major layouts the device DMAs straight in.
    qh = np.transpose(np.asarray(queries), (0, 2, 3, 1)).reshape(B * H, E, L)
    qh = np.ascontiguousarray(qh).astype(ml_dtypes.bfloat16)   # [64, E, L]
    kh = np.transpose(np.asarray(keys), (0, 2, 3, 1)).reshape(B * H, E, L)
    # [64, E, L] -> [64, 2, 8, E, 128]: half h gets s-chunks 2c+h
    kh = kh.reshape(B * H, E, NCHUNK // 2, 2, P)
    kh = np.ascontiguousarray(np.transpose(kh, (0, 3, 2, 1, 4))).astype(
        ml_dtypes.bfloat16)
    vh = np.transpose(np.asarray(values), (0, 2, 1, 3)).reshape(B * H, L, E)
    vh = np.ascontiguousarray(vh).astype(ml_dtypes.bfloat16)
    in_maps = [
        {"qt": qh[c * PAIRS : (c + 1) * PAIRS],
         "kt": kh[c * PAIRS : (c + 1) * PAIRS],
         "v": vh[c * PAIRS : (c + 1) * PAIRS]}
        for c in range(N_CORES)
    ]
    res = run_bass_kernel_spmd(nc, in_maps, list(range(N_CORES)),
                               trace=trace, **kw)
    # [64, E+1, L]: rows 0..63 unnormalized O^T, row 64 the softmax sums
    oh = np.concatenate([res.results[c]["o"] for c in range(N_CORES)], axis=0)
    onorm = oh[:, 0:E, :] / oh[:, E : E + 1, :]          # softmax divide
    out = np.transpose(onorm.reshape(B, H, E, L), (0, 3, 1, 2))
    return np.ascontiguousarray(out), res


def kernel(queries, keys, values):
    out, _ = run(queries, keys, values)
    return out
